# revision 1
# baseline (speedup 1.0000x reference)
"""GATv2Conv GNN message-passing kernel for 8 Trainium2 NeuronCores.

Strategy (self-contained — hardcoded for the 50000x116 / 800k-edge / 100-graph
problem shape, but parametrized from input shapes):
  * Host: append self-loops, sort edges by destination, shard contiguous graph
    ranges across 8 cores balancing edge counts, lay out per-edge source
    features [x[src]; ea; 1] as a bf16 stream (pure indexing/layout, no math).
  * Device (main SPMD program, per core):
      - xr table = x_T @ [Wr;br] per 128-node window (PE), stored to HBM bf16 (256B rows).
      - per 128-edge chunk: s = x_aug.T @ [Wl;We;bl] (PE) + xr[dst] via
        dma_gather + identity-matmul accumulate (PE); leaky via ACT Prelu
        (alpha honored on HW; sim falls back to relu_mm fold); logits =
        reduce(t*att) (DVE 2x, real-stride att); ex = exp (ACT) expanded to
        64 lanes by ACT copy so msg=gl*ex runs at DVE 2x; one-hot built by
        8x tensor_scalar is_equal (4x mode); scatter via one-hot matmul into
        per-window PSUM. exp/msg/scatter are skewed one group behind so the
        strict-FIFO ACT/DVE/PE queues never head-of-line block on each other.
      - per window: normalize by softmax denominator, accumulate per-graph
        sums of h and h^2 plus x-sums via one-hot matmuls into PSUM.
  * Device (tail SPMD program): global BN stats from per-core partials,
    BN-affine + residual fold, 2-layer MLP head. All cores compute the same
    tiny thing; core 0's output is used.
  * Host: reassemble [100, 2] output (pure indexing).
"""

import os
import numpy as np
import ml_dtypes

os.environ.setdefault("NEURON_RT_RESET_CORES", "1")
bf16 = ml_dtypes.bfloat16

P = 128
HEADS = 4
OUT_C = 16
D = 64
GSLOT = 16
GB_CHUNKS = 8  # chunks per dma_gather batch (ring limit: <2048 idxs)
NEG_SLOPE = 0.2
BN_EPS = 1e-5

_prog_cache = {}


# --------------------------------------------------------------------------
# host prep
# --------------------------------------------------------------------------

def _prep(inputs):
    x = np.asarray(inputs["x"], np.float32)
    ei = np.asarray(inputs["edge_index"], np.int32)
    ea = np.asarray(inputs["edge_attr"], np.float32)
    batch = np.asarray(inputs["batch"], np.int32)
    N, IN_C = x.shape
    E = ei.shape[1]
    G = int(batch.max()) + 1 if batch.size else 1
    G = max(G, 100) if N == 50000 else G  # fixed 100 graphs for this problem
    NC = 8
    CH = IN_C + 2           # x | ea | ones
    CHX = IN_C + 1          # x | ones

    src = np.concatenate([ei[0], np.arange(N, dtype=np.int32)])
    dst = np.concatenate([ei[1], np.arange(N, dtype=np.int32)])
    eav = np.concatenate([ea[:, 0], np.ones(N, np.float32)])
    order = np.argsort(dst, kind="stable")
    ss, ds, es = src[order], dst[order], eav[order]
    ET = ss.shape[0]

    nb = np.searchsorted(batch, np.arange(G + 1))          # node range per graph
    ecnt_g = np.bincount(batch[ds], minlength=G)            # edges per dst-graph
    csum = np.cumsum(ecnt_g)
    gb = [0]
    for k in range(1, NC):
        b = int(np.searchsorted(csum, ET * k / NC))
        gb.append(min(max(b, gb[-1] + 1), G - (NC - k)))
    gb.append(G)
    gb = np.array(gb, np.int64)

    cores = []
    Wmax, CPWmax = 1, 1
    for k in range(NC):
        g0, g1 = int(gb[k]), int(gb[k + 1])
        assert g1 - g0 <= GSLOT, f"core {k} has {g1-g0} graphs > {GSLOT}"
        n0, n1 = int(nb[g0]), int(nb[g1])
        e0, e1 = np.searchsorted(ds, [n0, n1])
        nloc = n1 - n0
        W = max(1, -(-nloc // P))
        rel = ds[e0:e1] - n0
        wofs = np.searchsorted(rel, np.arange(W + 1) * P)
        wcnt = np.diff(wofs)
        CPW = max(1, int(-(-wcnt.max() // P))) if wcnt.size else 1
        Wmax = max(Wmax, W)
        CPWmax = max(CPWmax, CPW)
        cores.append(dict(g0=g0, g1=g1, n0=n0, n1=n1, e0=int(e0), e1=int(e1),
                          rel=rel, wofs=wofs))

    W, CPW = Wmax, CPWmax
    T = W * CPW
    T8 = -(-T // GB_CHUNKS) * GB_CHUNKS
    L = T8 * P

    # shared weight prep
    Wl, bl = np.asarray(inputs["Wl"], np.float32), np.asarray(inputs["bl"], np.float32)
    Wr, br = np.asarray(inputs["Wr"], np.float32), np.asarray(inputs["br"], np.float32)
    We = np.asarray(inputs["We"], np.float32)
    att = np.asarray(inputs["att"], np.float32)
    waug = np.zeros((CH, 2 * D), np.float32)
    waug[:IN_C, :D] = Wl
    waug[:IN_C, D:] = Wl
    waug[IN_C, :D] = We[0]
    waug[CH - 1, :D] = bl
    waug[CH - 1, D:] = bl
    wr = np.concatenate([Wr, br[None, :]], 0)               # [CHX, 64]
    wres = np.concatenate([np.asarray(inputs["Wres"], np.float32),
                           np.asarray(inputs["bres"], np.float32)[None, :]], 0)
    w1 = np.concatenate([np.asarray(inputs["W1"], np.float32),
                         np.asarray(inputs["b1"], np.float32)[None, :]], 0)
    w2 = np.concatenate([np.asarray(inputs["W2"], np.float32),
                         np.asarray(inputs["b2"], np.float32)[None, :]], 0)
    attc = np.tile(att.reshape(1, D), (P, 8))
    iotac = np.tile(np.arange(P, dtype=np.float32), (P, 1))
    identc = np.eye(P, dtype=np.float32)
    nidentc = -np.eye(P, dtype=np.float32)
    misc = np.zeros((D, 8), np.float32)
    misc[:, 0] = np.asarray(inputs["gamma"], np.float32)
    misc[:, 1] = np.asarray(inputs["beta"], np.float32)
    misc[:, 2] = np.asarray(inputs["gat_bias"], np.float32)
    misc[:, 3] = BN_EPS

    cnt_g = (nb[1:] - nb[:-1]).astype(np.float32)

    shared = dict(
        waug=waug.astype(bf16), wr=wr.astype(bf16), wres=wres.astype(bf16),
        attc=attc.astype(bf16), iotac=iotac.astype(bf16),
        identc=identc.astype(bf16), nidentc=nidentc.astype(bf16),
        w1=w1.astype(bf16), w2=w2.astype(bf16), misc=misc,
    )

    in_maps = []
    for k in range(NC):
        c = cores[k]
        n0, n1, e0 = c["n0"], c["n1"], c["e0"]
        nloc = n1 - n0
        relc = c["rel"]
        wofs = c["wofs"]
        Wk = len(wofs) - 1

        sel = np.full(L, -1, np.int64)          # local edge position within core
        for w in range(Wk):
            cnt = wofs[w + 1] - wofs[w]
            if cnt:
                base = w * CPW * P
                sel[base:base + cnt] = wofs[w] + np.arange(cnt)
        valid = sel >= 0
        seli = np.where(valid, sel, 0)
        relv = relc[seli] if relc.size else np.zeros(L, np.int64)

        xga = np.zeros((CH, L), np.float32)
        xga[:IN_C] = np.where(valid, x[ss[e0 + seli]].T, 0.0)
        xga[IN_C] = np.where(valid, es[e0 + seli], 0.0)
        xga[CH - 1] = valid.astype(np.float32)

        pos_w = np.minimum(np.arange(L) // (CPW * P), W - 1)
        dstrel = np.where(valid, relv - pos_w * P, -1.0)
        dstrel = dstrel.astype(np.float32).reshape(T8, P).T    # [128, T8]

        idxv = np.where(valid, relv, 0).astype(np.int16)
        dsti = np.tile(idxv.reshape(-1, 16).T, (8, 1))          # [128, L/16]

        xt = np.zeros((CHX, W * P), np.float32)
        xt[:IN_C, :nloc] = x[n0:n1].T
        xt[IN_C, :nloc] = 1.0

        xnm_a = np.zeros((W * P, CHX), np.float32)
        xnm_a[:nloc, :IN_C] = x[n0:n1]
        xnm_a[:nloc, IN_C] = 1.0
        xnm = xnm_a.reshape(W, P, CHX).transpose(1, 0, 2).reshape(P, W * CHX)

        gm_a = np.zeros((W * P, 2 * GSLOT), np.float32)
        gsl = batch[n0:n1] - c["g0"]
        ar = np.arange(nloc)
        gm_a[ar, gsl] = 1.0
        gm_a[ar, GSLOT + gsl] = 1.0 / np.maximum(cnt_g[c["g0"]:c["g1"]], 1.0)[gsl]
        gmat = gm_a.reshape(W, P, 2 * GSLOT).transpose(1, 0, 2).reshape(P, W * 2 * GSLOT)

        m = dict(
            xga=xga.astype(bf16), dstrel=dstrel, dsti=dsti,
            xt=xt.astype(bf16), xnm=xnm.astype(bf16), gmat=gmat.astype(bf16),
        )
        for kk in ("waug", "wr", "wres", "attc", "iotac", "identc", "nidentc"):
            m[kk] = shared[kk]
        in_maps.append(m)

    meta = dict(N=N, IN_C=IN_C, CH=CH, CHX=CHX, G=G, NC=NC, W=W, CPW=CPW,
                T8=T8, gb=gb, cnt_g=cnt_g)
    return meta, in_maps, shared


# --------------------------------------------------------------------------
# bass programs
# --------------------------------------------------------------------------

def _build_main(meta, leaky_mode="relu_mm", debug=False, dbg_taps=False, ablate=()):
    import concourse.bacc as bacc
    import concourse.mybir as mybir
    import concourse.tile as tile

    F32 = mybir.dt.float32
    BF = mybir.dt.bfloat16
    I16 = mybir.dt.int16
    AL = mybir.AluOpType
    AF = mybir.ActivationFunctionType
    AX = mybir.AxisListType

    CH, CHX, W, CPW, T8 = meta["CH"], meta["CHX"], meta["W"], meta["CPW"], meta["T8"]
    NG = T8 // 8
    NB = T8 // GB_CHUNKS
    GS2 = 2 * GSLOT

    nc = bacc.Bacc(None, target_bir_lowering=False, debug=debug)

    t_xga = nc.dram_tensor("xga", [CH, T8 * P], BF, kind="ExternalInput")
    t_dstrel = nc.dram_tensor("dstrel", [P, T8], F32, kind="ExternalInput")
    t_dsti = nc.dram_tensor("dsti", [P, T8 * P // 16], I16, kind="ExternalInput")
    t_xt = nc.dram_tensor("xt", [CHX, W * P], BF, kind="ExternalInput")
    t_xnm = nc.dram_tensor("xnm", [P, W * CHX], BF, kind="ExternalInput")
    t_gmat = nc.dram_tensor("gmat", [P, W * GS2], BF, kind="ExternalInput")
    t_waug = nc.dram_tensor("waug", [CH, 2 * D], BF, kind="ExternalInput")
    t_wr = nc.dram_tensor("wr", [CHX, D], BF, kind="ExternalInput")
    t_wres = nc.dram_tensor("wres", [CHX, D], BF, kind="ExternalInput")
    t_attc = nc.dram_tensor("attc", [P, 8 * D], BF, kind="ExternalInput")
    t_iotac = nc.dram_tensor("iotac", [P, P], BF, kind="ExternalInput")
    t_id = nc.dram_tensor("identc", [P, P], BF, kind="ExternalInput")
    t_nid = nc.dram_tensor("nidentc", [P, P], BF, kind="ExternalInput")

    o_s = nc.dram_tensor("o_s", [P, 1], F32, kind="ExternalOutput")
    o_hdiv = nc.dram_tensor("o_hdiv", [D, GSLOT], F32, kind="ExternalOutput")
    o_res = nc.dram_tensor("o_res", [D, GSLOT], F32, kind="ExternalOutput")

    xrtab = nc.dram_tensor("xrtab", [W * P, P], BF)
    if dbg_taps:
        d_t = nc.dram_tensor("d_t", [P, 8, D], F32, kind="ExternalOutput")
        d_lg = nc.dram_tensor("d_lg", [P, 8, HEADS], F32, kind="ExternalOutput")
        d_msg = nc.dram_tensor("d_msg", [P, 8, D + HEADS], F32, kind="ExternalOutput")
        d_oh = nc.dram_tensor("d_oh", [P, 8, P], F32, kind="ExternalOutput")
        d_gr = nc.dram_tensor("d_gr", [P, 8, D], F32, kind="ExternalOutput")
        d_win = nc.dram_tensor("d_win", [P, D + HEADS], F32, kind="ExternalOutput")
        d_s = nc.dram_tensor("d_s", [P, 8, D], F32, kind="ExternalOutput")
        d_gl = nc.dram_tensor("d_gl", [P, 8, D], F32, kind="ExternalOutput")

    with tile.TileContext(nc) as tc:
        with tc.tile_pool(name="cst", bufs=1) as cst, \
             tc.tile_pool(name="sgl", bufs=2, space="PSUM") as ps_sgl_pool, \
             tc.tile_pool(name="win", bufs=2, space="PSUM") as ps_win_pool, \
             tc.tile_pool(name="acc", bufs=1, space="PSUM") as ps_acc_pool, \
             tc.tile_pool(name="xsm", bufs=1, space="PSUM") as ps_xsm_pool, \
             tc.tile_pool(name="str", bufs=4) as strm, \
             tc.tile_pool(name="gat", bufs=3) as gatp, \
             tc.tile_pool(name="wrk", bufs=3) as wrk:

            def load_const(t, shape, dtype):
                s = cst.tile(shape, dtype, tag=t.name)
                nc.sync.dma_start(s[:], t[:])
                return s

            # phase-B-critical consts first: HWDGE drains in FIFO order, so
            # xt/wr must not queue behind the 1.9MB dsti load
            xt_t = load_const(t_xt, [CHX, W * P], BF)
            wr_t = load_const(t_wr, [CHX, D], BF)
            xnm_t = load_const(t_xnm, [P, W * CHX], BF)
            gmat_t = load_const(t_gmat, [P, W * GS2], BF)
            waug_t = load_const(t_waug, [CH, 2 * D], BF)
            iotac_t = load_const(t_iotac, [P, P], BF)
            id_t = load_const(t_id, [P, P], BF)
            dstrel_t = load_const(t_dstrel, [P, T8], F32)
            dsti_t = load_const(t_dsti, [P, T8 * P // 16], I16)
            attc_t = load_const(t_attc, [P, 8 * D], BF)
            wres_t = load_const(t_wres, [CHX, D], BF)
            nid_t = load_const(t_nid, [P, P], BF)

            xnm_v = xnm_t[:].rearrange("p (w c) -> p w c", w=W)
            gmat_v = gmat_t[:].rearrange("p (w g) -> p w g", w=W)

            ps_stats = ps_acc_pool.tile([P, GS2], F32, tag="stats")
            ps_xsum = ps_xsm_pool.tile([CHX, GS2], F32, tag="xsum")

            # phase B: xr table (batched 8 windows per psum bank) + x sums
            W8 = -(-W // 8)
            for w8 in range(W8):
                nw = min(8, W - w8 * 8)
                ps_xr = ps_win_pool.tile([P, 8, D], F32, tag="win",
                                         name=f"xr{w8}")
                for j in range(nw):
                    w = w8 * 8 + j
                    nc.tensor.matmul(ps_xr[:, j, :],
                                     xt_t[:, w * P:(w + 1) * P], wr_t[:],
                                     start=(j == 0), stop=True,
                                     skip_group_check=True)
                sb_xr = wrk.tile([P, 8, P], BF, tag="xrw", name=f"xrw{w8}")
                nc.vector.memset(sb_xr[:, :, D:P], 0.0)
                nc.scalar.activation(sb_xr[:, 0:nw, 0:D], ps_xr[:, 0:nw, :],
                                     AF.Copy)
                nc.sync.dma_start(
                    xrtab[w8 * 8 * P:w8 * 8 * P + nw * P, :].rearrange(
                        "(w p) f -> p w f", p=P),
                    sb_xr[:, 0:nw, :])
            for w in range(W):
                nc.tensor.matmul(ps_xsum[:], xnm_v[:, w, :], gmat_v[:, w, :],
                                 start=(w == 0), stop=(w == W - 1),
                                 skip_group_check=True)

            # phase C: edge loop (scatter matmuls skewed one group behind so
            # PE never stalls on the DVE logits chain)
            win_tiles = {}
            gr_tile = None
            pend = []

            def emit_scatter(gq, oh_q, msg_q, gl_q, lg_q):
                sb_exq = wrk.tile([P, 8, D], BF, tag="exq", name=f"exq{gq}")
                nc.scalar.activation(
                    sb_exq[:].rearrange("p c (h k) -> p c h k", k=OUT_C),
                    msg_q[:, :, D:D + HEADS].unsqueeze(3).to_broadcast(
                        [P, 8, HEADS, OUT_C]),
                    AF.Copy)
                nc.vector.tensor_tensor(
                    out=msg_q[:, :, 0:D], in0=gl_q[:], in1=sb_exq[:],
                    op=AL.mult)
                flush = []
                for c8 in range(8):
                    c = gq * 8 + c8
                    w = min(c // CPW, W - 1)
                    if w not in win_tiles:
                        win_tiles[w] = ps_win_pool.tile([P, D + HEADS], F32,
                                                        tag="win", name=f"win{gq}_{w}")
                    first = (c % CPW == 0) and c < W * CPW
                    last = (c == (w + 1) * CPW - 1) if w < W - 1 else (c == T8 - 1)
                    nc.tensor.matmul(win_tiles[w][:], oh_q[:, c8, :],
                                     msg_q[:, c8, :], start=first, stop=last,
                                     skip_group_check=True)
                    if last:
                        flush.append(w)
                return flush

            def do_flush(flush):
                for w in flush:
                    ps_w = win_tiles.pop(w)
                    sb_den = wrk.tile([P, HEADS], F32, tag="den", name=f"den{w}")
                    nc.vector.tensor_scalar(sb_den[:], ps_w[:, D:D + HEADS],
                                            1e-20, None, AL.add)
                    sb_rd = wrk.tile([P, HEADS], F32, tag="rd", name=f"rd{w}")
                    nc.vector.reciprocal(sb_rd[:], sb_den[:])
                    sb_hh2 = wrk.tile([P, 2 * D], BF, tag="hh2", name=f"hh2{w}")
                    nc.vector.tensor_tensor(
                        out=sb_hh2[:, 0:D].rearrange("p (h k) -> p h k", k=OUT_C),
                        in0=ps_w[:, 0:D].rearrange("p (h k) -> p h k", k=OUT_C),
                        in1=sb_rd[:].unsqueeze(2).to_broadcast([P, HEADS, OUT_C]),
                        op=AL.mult)
                    nc.scalar.activation(sb_hh2[:, D:2 * D], sb_hh2[:, 0:D],
                                         AF.Square)
                    nc.tensor.matmul(ps_stats[:], sb_hh2[:], gmat_v[:, w, :],
                                     start=(w == 0), stop=(w == W - 1),
                                     skip_group_check=True)

            for g in range(NG):
                xga_t = strm.tile([CH, 8 * P], BF, tag="xga")
                nc.sync.dma_start(xga_t[:], t_xga[:, g * 8 * P:(g + 1) * 8 * P])
                if g % (GB_CHUNKS // 8) == 0:
                    b = g // (GB_CHUNKS // 8)
                    gr_tile = gatp.tile([P, GB_CHUNKS, P], BF, tag="gr")
                    nidx = GB_CHUNKS * P
                    nc.gpsimd.dma_gather(
                        out_ap=gr_tile[:],
                        in_ap=xrtab[:],
                        idxs_ap=dsti_t[:, b * (nidx // 16):(b + 1) * (nidx // 16)],
                        num_idxs=nidx, num_idxs_reg=nidx, elem_size=P)

                ps_sgl = ps_sgl_pool.tile([P, 8, 2 * D], F32, tag="sgl")
                for c8 in range(8):
                    nc.tensor.matmul(ps_sgl[:, c8, :],
                                     xga_t[:, c8 * P:(c8 + 1) * P], waug_t[:],
                                     start=(c8 % 4 == 0), stop=True,
                                     skip_group_check=True)
                goff = (g % (GB_CHUNKS // 8)) * 8
                if "grmm" not in ablate:
                    for c8 in range(8):
                        nc.tensor.matmul(ps_sgl[:, c8, 0:D], id_t[:],
                                         gr_tile[:, goff + c8, 0:D],
                                         start=False, stop=True, skip_group_check=True)

                if dbg_taps and g == 0:
                    dsf = wrk.tile([P, 8, D], F32, tag="dsf")
                    nc.scalar.activation(dsf[:], ps_sgl[:, :, 0:D], AF.Copy)
                    nc.sync.dma_start(d_s[:], dsf[:])
                    dglf = wrk.tile([P, 8, D], F32, tag="dglf")
                    nc.scalar.activation(dglf[:], ps_sgl[:, :, D:2 * D], AF.Copy)
                    nc.sync.dma_start(d_gl[:], dglf[:])

                sb_t = wrk.tile([P, 8, D], BF, tag="t")
                if leaky_mode == "prelu":
                    nc.scalar.activation(sb_t[:], ps_sgl[:, :, 0:D], AF.Prelu,
                                         alpha=NEG_SLOPE)
                else:
                    sb_r2 = wrk.tile([P, 8, D], BF, tag="r2")
                    nc.scalar.activation(sb_r2[:], ps_sgl[:, :, 0:D], AF.Relu,
                                         scale=-(1.0 - NEG_SLOPE))
                    for c8 in range(8):
                        nc.tensor.matmul(ps_sgl[:, c8, 0:D], id_t[:],
                                         sb_r2[:, c8, :],
                                         start=False, stop=True,
                                         skip_group_check=True)
                    nc.scalar.activation(sb_t[:], ps_sgl[:, :, 0:D], AF.Copy)
                if pend:
                    _, _, pmsg, _, plg = pend[-1]
                    nc.scalar.activation(pmsg[:, :, D:D + HEADS], plg[:], AF.Exp)
                sb_gl = wrk.tile([P, 8, D], BF, tag="gl")
                nc.scalar.activation(sb_gl[:], ps_sgl[:, :, D:2 * D], AF.Copy)

                sb_u = wrk.tile([P, 8, D], BF, tag="u")
                nc.vector.tensor_tensor(
                    out=sb_u[:], in0=sb_t[:],
                    in1=attc_t[:].rearrange("p (c f) -> p c f", c=8),
                    op=AL.mult)
                sb_lg = wrk.tile([P, 8, HEADS], F32, tag="lg")
                nc.vector.tensor_reduce(
                    out=sb_lg[:],
                    in_=sb_u[:].rearrange("p c (h k) -> p c h k", k=OUT_C),
                    axis=AX.X, op=AL.add)
                sb_msg = wrk.tile([P, 8, D + HEADS], BF, tag="msg")

                oh_t = wrk.tile([P, 8, P], BF, tag="oh")
                if "oh" not in ablate:
                    for c8 in range(8):
                        nc.vector.tensor_scalar(
                            oh_t[:, c8, :], iotac_t[:],
                            dstrel_t[:, g * 8 + c8:g * 8 + c8 + 1], None,
                            AL.is_equal)

                if dbg_taps and g == 0:
                    dtf = wrk.tile([P, 8, D], F32, tag="dtf")
                    nc.vector.tensor_copy(dtf[:], sb_t[:])
                    nc.sync.dma_start(d_t[:], dtf[:])
                    nc.sync.dma_start(d_lg[:], sb_lg[:])
                    dmf = wrk.tile([P, 8, D + HEADS], F32, tag="dmf")
                    nc.vector.tensor_copy(dmf[:], sb_msg[:])
                    nc.sync.dma_start(d_msg[:], dmf[:])
                    dof = wrk.tile([P, 8, P], F32, tag="dof")
                    nc.vector.tensor_copy(dof[:], oh_t[:])
                    nc.sync.dma_start(d_oh[:], dof[:])
                    dgf = wrk.tile([P, 8, D], F32, tag="dgf")
                    nc.vector.tensor_copy(dgf[:], gr_tile[:, goff:goff + 8, 0:D])
                    nc.sync.dma_start(d_gr[:], dgf[:])

                pend.append((g, oh_t, sb_msg, sb_gl, sb_lg))
                if len(pend) > 1:
                    do_flush(emit_scatter(*pend.pop(0)))

            while pend:
                _, _, pmsg, _, plg = pend[0]
                nc.scalar.activation(pmsg[:, :, D:D + HEADS], plg[:], AF.Exp)
                do_flush(emit_scatter(*pend.pop(0)))

            # phase D: outputs
            sb_sloc = wrk.tile([P, 1], F32, tag="sloc")
            nc.vector.tensor_reduce(out=sb_sloc[:], in_=ps_stats[:, 0:GSLOT],
                                    axis=AX.X, op=AL.add)
            nc.sync.dma_start(o_s[:], sb_sloc[:])
            sb_hdiv = wrk.tile([D, GSLOT], F32, tag="hdiv")
            nc.scalar.activation(sb_hdiv[:], ps_stats[0:D, GSLOT:GS2], AF.Copy)
            nc.sync.dma_start(o_hdiv[:], sb_hdiv[:])
            sb_xdiv = wrk.tile([CHX, GSLOT], BF, tag="xdiv")
            nc.scalar.activation(sb_xdiv[:], ps_xsum[:, GSLOT:GS2], AF.Copy)
            ps_res = ps_sgl_pool.tile([D, GSLOT], F32, tag="sgl")
            nc.tensor.matmul(ps_res[:], wres_t[:], sb_xdiv[:], start=True,
                             stop=True, skip_group_check=True)
            sb_res = wrk.tile([D, GSLOT], F32, tag="res")
            nc.scalar.activation(sb_res[:], ps_res[:], AF.Copy)
            nc.sync.dma_start(o_res[:], sb_res[:])

    nc.compile()
    return nc


def _build_tail(meta, debug=False):
    import concourse.bacc as bacc
    import concourse.mybir as mybir
    import concourse.tile as tile

    F32 = mybir.dt.float32
    BF = mybir.dt.bfloat16
    AL = mybir.AluOpType
    AF = mybir.ActivationFunctionType
    AX = mybir.AxisListType

    N = meta["N"]
    NC = meta["NC"]
    GALL = NC * GSLOT  # 128

    FPK = 2 * NC + 2 * GALL + 8
    nc = bacc.Bacc(None, target_bir_lowering=False, debug=debug)
    t_fpk = nc.dram_tensor("t_fpk", [D, FPK], F32, kind="ExternalInput")
    t_wpk = nc.dram_tensor("t_wpk", [D + 1, D + 2], BF, kind="ExternalInput")
    t_out = nc.dram_tensor("t_out", [2, GALL], F32, kind="ExternalOutput")

    with tile.TileContext(nc) as tc:
        with tc.tile_pool(name="sb", bufs=1) as sb, \
             tc.tile_pool(name="ps", bufs=2, space="PSUM") as ps:
            fpk = sb.tile([D, FPK], F32, tag="fpk")
            nc.sync.dma_start(fpk[:], t_fpk[:])
            wpk = sb.tile([D + 1, D + 2], BF, tag="wpk")
            nc.sync.dma_start(wpk[:], t_wpk[:])
            s8 = fpk[:, 0:2 * NC]
            hdiv = fpk[:, 2 * NC:2 * NC + GALL]
            res = fpk[:, 2 * NC + GALL:2 * NC + 2 * GALL]
            misc = fpk[:, 2 * NC + 2 * GALL:FPK]
            w1 = wpk[:, 0:D]
            w2 = wpk[:, D:D + 2]

            sh = sb.tile([D, 1], F32, tag="sh")
            nc.vector.tensor_reduce(out=sh[:], in_=s8[:, 0:NC], axis=AX.X, op=AL.add)
            sh2 = sb.tile([D, 1], F32, tag="sh2")
            nc.vector.tensor_reduce(out=sh2[:], in_=s8[:, NC:2 * NC], axis=AX.X,
                                    op=AL.add)
            mu = sb.tile([D, 1], F32, tag="mu")
            nc.scalar.activation(mu[:], sh[:], AF.Copy, scale=1.0 / N)
            e2 = sb.tile([D, 1], F32, tag="e2")
            nc.scalar.activation(e2[:], sh2[:], AF.Copy, scale=1.0 / N)
            mu2 = sb.tile([D, 1], F32, tag="mu2")
            nc.scalar.activation(mu2[:], mu[:], AF.Square)
            var = sb.tile([D, 1], F32, tag="var")
            nc.vector.tensor_tensor(out=var[:], in0=e2[:], in1=mu2[:], op=AL.subtract)
            sd = sb.tile([D, 1], F32, tag="sd")
            nc.scalar.activation(sd[:], var[:], AF.Sqrt, bias=misc[:, 3:4])
            rsd = sb.tile([D, 1], F32, tag="rsd")
            nc.vector.reciprocal(rsd[:], sd[:])
            A = sb.tile([D, 1], F32, tag="A")
            nc.vector.tensor_tensor(out=A[:], in0=misc[:, 0:1], in1=rsd[:], op=AL.mult)
            tmp2 = sb.tile([D, 1], F32, tag="tmp2")
            nc.vector.tensor_tensor(out=tmp2[:], in0=A[:], in1=mu[:], op=AL.mult)
            B = sb.tile([D, 1], F32, tag="B")
            nc.vector.tensor_tensor(out=B[:], in0=misc[:, 1:2], in1=tmp2[:],
                                    op=AL.subtract)

            pooled = sb.tile([D, GALL], F32, tag="pooled")
            nc.vector.tensor_scalar(pooled[:], hdiv, A[:], B[:], AL.mult, AL.add)
            zr = sb.tile([D + 1, GALL], BF, tag="zr")
            nc.vector.memset(zr[D:D + 1, :], 1.0)
            nc.vector.tensor_tensor(out=zr[0:D, :], in0=pooled[:], in1=res,
                                    op=AL.add)
            ps_z = ps.tile([D, GALL], F32, tag="z")
            nc.tensor.matmul(ps_z[:], w1, zr[:], start=True, stop=True)
            z2 = sb.tile([D + 1, GALL], BF, tag="z2")
            nc.vector.memset(z2[D:D + 1, :], 1.0)
            nc.scalar.activation(z2[0:D, :], ps_z[:], AF.Relu)
            ps_o = ps.tile([2, GALL], F32, tag="o")
            nc.tensor.matmul(ps_o[:], w2, z2[:], start=True, stop=True)
            sb_o = sb.tile([2, GALL], F32, tag="out")
            nc.scalar.activation(sb_o[:], ps_o[:], AF.Copy)
            nc.sync.dma_start(t_out[:], sb_o[:])

    nc.compile()
    return nc


# --------------------------------------------------------------------------
# entry point
# --------------------------------------------------------------------------

def _run_sim(nc, in_maps, out_names):
    from concourse.bass_interp import CoreSim
    outs = []
    for m in in_maps:
        sim = CoreSim(nc, require_finite=False, require_nnan=False)
        for name, arr in m.items():
            sim.tensor(name)[:] = arr
        sim.simulate()
        outs.append({n: np.array(sim.tensor(n)) for n in out_names})
    return outs


def kernel(**inputs):
    meta, in_maps, shared = _prep(inputs)
    key = ("main", meta["CH"], meta["W"], meta["CPW"], meta["T8"], _LEAKY_MODE)
    if key not in _prog_cache:
        _prog_cache[key] = _build_main(meta, leaky_mode=_LEAKY_MODE,
                                       debug=(_RUN_MODE == "sim"))
    nc_main = _prog_cache[key]
    tkey = ("tail", meta["N"])
    if tkey not in _prog_cache:
        _prog_cache[tkey] = _build_tail(meta, debug=(_RUN_MODE == "sim"))
    nc_tail = _prog_cache[tkey]

    NC = meta["NC"]
    core_ids = list(range(NC))
    global LAST_EXEC_NS
    if _RUN_MODE == "sim":
        res1 = _run_sim(nc_main, in_maps, ["o_s", "o_hdiv", "o_res"])
        LAST_EXEC_NS = [None]
    else:
        from concourse.bass_utils import run_bass_kernel_spmd
        import time as _time
        _t0 = _time.time()
        r1 = run_bass_kernel_spmd(nc_main, in_maps, core_ids, **_RUN_KW)
        _t1 = _time.time()
        res1 = r1.results
        LAST_EXEC_NS = [getattr(r1, "exec_time_ns", None) or int((_t1 - _t0) * 1e9)]

    s8 = np.zeros((D, 2 * NC), np.float32)
    hdiv = np.zeros((D, NC * GSLOT), np.float32)
    resm = np.zeros((D, NC * GSLOT), np.float32)
    for k in range(NC):
        sk = res1[k]["o_s"]
        s8[:, k] = sk[0:D, 0]
        s8[:, NC + k] = sk[D:2 * D, 0]
        hdiv[:, k * GSLOT:(k + 1) * GSLOT] = res1[k]["o_hdiv"]
        resm[:, k * GSLOT:(k + 1) * GSLOT] = res1[k]["o_res"]

    fpk = np.concatenate([s8, hdiv, resm, shared["misc"]], axis=1).astype(np.float32)
    wpk = np.concatenate([shared["w1"], shared["w2"]], axis=1)
    tail_map = dict(t_fpk=fpk, t_wpk=wpk)
    if _RUN_MODE == "sim":
        res2 = _run_sim(nc_tail, [tail_map], ["t_out"])
        LAST_EXEC_NS.append(None)
    else:
        from concourse.bass_utils import run_bass_kernel_spmd
        import time as _time
        _t0 = _time.time()
        r2 = run_bass_kernel_spmd(nc_tail, [tail_map] * NC, core_ids,
                                  **_RUN_KW_TAIL)
        _t1 = _time.time()
        res2 = r2.results
        LAST_EXEC_NS.append(getattr(r2, "exec_time_ns", None) or int((_t1 - _t0) * 1e9))
    t_out = res2[0]["t_out"]

    G = meta["G"]
    gb = meta["gb"]
    out = np.zeros((G, 2), np.float32)
    for g in range(G):
        k = int(np.searchsorted(gb, g, side="right")) - 1
        slot = g - int(gb[k])
        out[g] = t_out[:, k * GSLOT + slot]
    return out


_LEAKY_MODE = "prelu"
_RUN_MODE = "hw"
_RUN_KW = {}
_RUN_KW_TAIL = {}
LAST_EXEC_NS = None



# revision 6
# speedup vs baseline: 6.5623x; 6.5623x over previous
"""GATv2Conv GNN message-passing kernel for 8 Trainium2 NeuronCores.

Strategy (self-contained — hardcoded for the 50000x116 / 800k-edge / 100-graph
problem shape, but parametrized from input shapes):
  * Host (pure indexing/layout, no math): append self-loops, sort edges by
    destination, shard contiguous graph ranges across 8 cores balancing edge
    counts, build compact per-edge index/scalar streams (src/dst gather ids,
    window-relative dst, src parity, edge_attr) and window-major node features.
  * Device (single SPMD program, per core):
      - phase B: per 128-node window, PE-transpose the node features, project
        [xl|xr] = x @ [Wl|Wr] (PE), write a local [node -> xl|xr] table (256B
        rows) for the xr[dst] gather and a pack-2 xl half-slab; also accumulate
        per-graph x sums (PE one-hot matmul) for the residual term.
      - AllGather the xl half-slabs -> global pack-2 xl table (26k rows, so
        int16 gather indices cover all 50k nodes; the packed column half is
        selected per-edge by a parity stream).
      - phase C edge loop, 8-chunk groups: dma_gather xl[src] (global table,
        queue 1) and xr[dst] (local table, queue 0); s = xl+xr+ea*We (DVE);
        leaky via ACT Prelu; logits = reduce(s*att) (DVE, real-stride att);
        exp (ACT) expanded to 64 lanes by ACT copy; msg = xl_sel*ex (DVE);
        one-hot scatter via PE matmul into per-window PSUM. exp/msg/scatter
        are skewed one group behind so the strict-FIFO ACT/DVE/PE queues
        never head-of-line block on each other.
      - per window: normalize by softmax denominator, accumulate per-graph
        sums of h and h^2 plus x-sums into PSUM.
      - tail (fused): AllGather the tiny per-core partials (BN sums, pooled h,
        residual), then every core computes global BN stats + affine +
        residual + 2-layer MLP head redundantly; core 0's output is used.
  * Host: reassemble [100, 2] output (pure indexing).
"""

import os
import numpy as np
import ml_dtypes

os.environ.setdefault("NEURON_RT_RESET_CORES", "1")
bf16 = ml_dtypes.bfloat16

P = 128
HEADS = 4
OUT_C = 16
D = 64
GSLOT = 16
NEG_SLOPE = 0.2
BN_EPS = 1e-5

_prog_cache = {}


# --------------------------------------------------------------------------
# host prep
# --------------------------------------------------------------------------

def _prep(inputs):
    x = np.asarray(inputs["x"], np.float32)
    ei = np.asarray(inputs["edge_index"], np.int32)
    ea = np.asarray(inputs["edge_attr"], np.float32)
    batch = np.asarray(inputs["batch"], np.int32)
    N, IN_C = x.shape
    E = ei.shape[1]
    G = int(batch.max()) + 1 if batch.size else 1
    G = max(G, 100) if N == 50000 else G  # fixed 100 graphs for this problem
    NC = 8
    CHX = IN_C + 1          # x | ones

    src = np.concatenate([ei[0], np.arange(N, dtype=np.int32)])
    dst = np.concatenate([ei[1], np.arange(N, dtype=np.int32)])
    eav_full = np.concatenate([ea[:, 0], np.ones(N, np.float32)])
    order = np.argsort(dst, kind="stable")
    ss, ds, es = src[order], dst[order], eav_full[order]
    ET = ss.shape[0]

    nb = np.searchsorted(batch, np.arange(G + 1))          # node range per graph
    ecnt_g = np.bincount(batch[ds], minlength=G)            # edges per dst-graph
    csum = np.cumsum(ecnt_g)
    gb = [0]
    for k in range(1, NC):
        b = int(np.searchsorted(csum, ET * k / NC))
        gb.append(min(max(b, gb[-1] + 1), G - (NC - k)))
    gb.append(G)
    gb = np.array(gb, np.int64)

    cores = []
    Wmax, CPWmax = 1, 1
    for k in range(NC):
        g0, g1 = int(gb[k]), int(gb[k + 1])
        assert g1 - g0 <= GSLOT, f"core {k} has {g1-g0} graphs > {GSLOT}"
        n0, n1 = int(nb[g0]), int(nb[g1])
        e0, e1 = np.searchsorted(ds, [n0, n1])
        nloc = n1 - n0
        W = max(1, -(-nloc // P))
        rel = ds[e0:e1] - n0
        wofs = np.searchsorted(rel, np.arange(W + 1) * P)
        wcnt = np.diff(wofs)
        CPW = max(1, int(-(-wcnt.max() // P))) if wcnt.size else 1
        Wmax = max(Wmax, W)
        CPWmax = max(CPWmax, CPW)
        cores.append(dict(g0=g0, g1=g1, n0=n0, n1=n1, e0=int(e0), e1=int(e1),
                          rel=rel, wofs=wofs))

    W, CPW = Wmax, CPWmax
    T = W * CPW
    T8 = -(-T // 8) * 8
    L = T8 * P
    SLH = W * (P // 2)      # half-slab rows per core in the global xl table
    assert NC * SLH < 32768, "global xl table must fit int16 gather indices"
    nodestart = np.array([c["n0"] for c in cores], np.int64)

    # shared weight prep
    Wl, bl = np.asarray(inputs["Wl"], np.float32), np.asarray(inputs["bl"], np.float32)
    Wr, br = np.asarray(inputs["Wr"], np.float32), np.asarray(inputs["br"], np.float32)
    We = np.asarray(inputs["We"], np.float32)
    att = np.asarray(inputs["att"], np.float32)
    wlr = np.zeros((CHX, 2 * D), np.float32)
    wlr[:IN_C, :D] = Wl
    wlr[:IN_C, D:] = Wr
    wlr[IN_C, :D] = bl
    wlr[IN_C, D:] = br
    wres = np.concatenate([np.asarray(inputs["Wres"], np.float32),
                           np.asarray(inputs["bres"], np.float32)[None, :]], 0)
    w1 = np.concatenate([np.asarray(inputs["W1"], np.float32),
                         np.asarray(inputs["b1"], np.float32)[None, :]], 0)
    w2 = np.concatenate([np.asarray(inputs["W2"], np.float32),
                         np.asarray(inputs["b2"], np.float32)[None, :]], 0)
    attc = np.tile(att.reshape(1, D), (P, 8))
    webc = np.tile(We.reshape(1, D), (P, 1))
    iotac = np.tile(np.arange(P, dtype=np.float32), (P, 1))
    identc = np.eye(P, dtype=np.float32)
    misc = np.zeros((D, 8), np.float32)
    misc[:, 0] = np.asarray(inputs["gamma"], np.float32)
    misc[:, 1] = np.asarray(inputs["beta"], np.float32)
    misc[:, 3] = BN_EPS

    cnt_g = (nb[1:] - nb[:-1]).astype(np.float32)

    shared = dict(
        wlr=wlr.astype(bf16), wres=wres.astype(bf16),
        attc=attc.astype(bf16), webc=webc.astype(bf16),
        iotac=iotac.astype(bf16), identc=identc.astype(bf16),
        w1=w1.astype(bf16), w2=w2.astype(bf16), misc=misc,
    )

    in_maps = []
    for k in range(NC):
        c = cores[k]
        n0, n1, e0 = c["n0"], c["n1"], c["e0"]
        nloc = n1 - n0
        relc = c["rel"]
        wofs = c["wofs"]
        Wk = len(wofs) - 1

        sel = np.full(L, -1, np.int64)          # local edge position within core
        for w in range(Wk):
            cnt = wofs[w + 1] - wofs[w]
            if cnt:
                base = w * CPW * P
                sel[base:base + cnt] = wofs[w] + np.arange(cnt)
        valid = sel >= 0
        seli = np.where(valid, sel, 0)
        relv = relc[seli] if relc.size else np.zeros(L, np.int64)

        # global src -> (pack-2 row, column half) in the AllGathered xl table
        srcg = ss[e0 + seli] if relc.size else np.zeros(L, np.int64)
        own = np.searchsorted(nodestart, srcg, side="right") - 1
        loc = srcg - nodestart[own]
        row = own * SLH + (loc % SLH)
        half = loc // SLH
        srci_v = np.where(valid, row, 0).astype(np.int16)
        parv = np.where(valid, half, 0).astype(np.float32)
        eav = np.where(valid, es[e0 + seli], 0.0).astype(np.float32)

        pos_w = np.minimum(np.arange(L) // (CPW * P), W - 1)
        dstrel = np.where(valid, relv - pos_w * P, -1.0)
        dstrel = dstrel.astype(np.float32).reshape(T8, P).T    # [128, T8]
        par = parv.reshape(T8, P).T
        eac = eav.reshape(T8, P).T

        idxv = np.where(valid, relv, 0).astype(np.int16)
        dsti = idxv.reshape(-1, 16).T                          # [16, L/16]
        srci = srci_v.reshape(-1, 16).T                        # [16, L/16]

        xnm_a = np.zeros((W * P, CHX), np.float32)
        xnm_a[:nloc, :IN_C] = x[n0:n1]
        xnm_a[:nloc, IN_C] = 1.0
        xnm = xnm_a.reshape(W, P, CHX).transpose(1, 0, 2).reshape(P, W * CHX)

        gm_a = np.zeros((W * P, 2 * GSLOT), np.float32)
        gsl = batch[n0:n1] - c["g0"]
        ar = np.arange(nloc)
        gm_a[ar, gsl] = 1.0
        gm_a[ar, GSLOT + gsl] = 1.0 / np.maximum(cnt_g[c["g0"]:c["g1"]], 1.0)[gsl]
        gmat = gm_a.reshape(W, P, 2 * GSLOT).transpose(1, 0, 2).reshape(P, W * 2 * GSLOT)

        m = dict(
            xnm=xnm.astype(bf16), gmat=gmat.astype(bf16),
            dstrel=dstrel.astype(bf16), par=par.astype(bf16),
            eac=eac.astype(bf16), dsti=dsti, srci=srci,
        )
        m.update(shared)
        in_maps.append(m)

    meta = dict(N=N, IN_C=IN_C, CHX=CHX, G=G, NC=NC, W=W, CPW=CPW,
                T8=T8, SLH=SLH, gb=gb)
    return meta, in_maps


# --------------------------------------------------------------------------
# bass program (single launch: GAT + BN/residual/pool partials + fused tail)
# --------------------------------------------------------------------------

def _build(meta, debug=False):
    import concourse.bacc as bacc
    import concourse.mybir as mybir
    import concourse.tile as tile

    F32 = mybir.dt.float32
    BF = mybir.dt.bfloat16
    I16 = mybir.dt.int16
    AL = mybir.AluOpType
    AF = mybir.ActivationFunctionType
    AX = mybir.AxisListType

    N = meta["N"]
    CHX, W, CPW, T8 = meta["CHX"], meta["W"], meta["CPW"], meta["T8"]
    NC = meta["NC"]
    SLH = meta["SLH"]
    NG = T8 // 8
    GS2 = 2 * GSLOT
    GALL = NC * GSLOT
    L16 = T8 * 8            # idx stream cols ([16, L16] wrapped format)
    FC = 2 + 2 * GSLOT      # per-core partials row: [sum_h, sum_h2, hdiv, res]

    nc = bacc.Bacc(None, target_bir_lowering=False, debug=debug,
                   num_devices=NC, num_swdge_queues=2)

    t_xnm = nc.dram_tensor("xnm", [P, W * CHX], BF, kind="ExternalInput")
    t_gmat = nc.dram_tensor("gmat", [P, W * GS2], BF, kind="ExternalInput")
    t_dstrel = nc.dram_tensor("dstrel", [P, T8], BF, kind="ExternalInput")
    t_par = nc.dram_tensor("par", [P, T8], BF, kind="ExternalInput")
    t_eac = nc.dram_tensor("eac", [P, T8], BF, kind="ExternalInput")
    t_dsti = nc.dram_tensor("dsti", [16, L16], I16, kind="ExternalInput")
    t_srci = nc.dram_tensor("srci", [16, L16], I16, kind="ExternalInput")
    t_wlr = nc.dram_tensor("wlr", [CHX, 2 * D], BF, kind="ExternalInput")
    t_wres = nc.dram_tensor("wres", [CHX, D], BF, kind="ExternalInput")
    t_attc = nc.dram_tensor("attc", [P, 8 * D], BF, kind="ExternalInput")
    t_webc = nc.dram_tensor("webc", [P, D], BF, kind="ExternalInput")
    t_iotac = nc.dram_tensor("iotac", [P, P], BF, kind="ExternalInput")
    t_id = nc.dram_tensor("identc", [P, P], BF, kind="ExternalInput")
    t_misc = nc.dram_tensor("misc", [D, 8], F32, kind="ExternalInput")
    t_w1 = nc.dram_tensor("w1", [D + 1, D], BF, kind="ExternalInput")
    t_w2 = nc.dram_tensor("w2", [D + 1, 2], BF, kind="ExternalInput")

    t_out = nc.dram_tensor("t_out", [2, GALL], F32, kind="ExternalOutput")

    xrtab = nc.dram_tensor("xrtab", [W * P, P], BF)            # local [xl|xr]
    xlslab = nc.dram_tensor("xlslab", [SLH, P], BF)            # local pack-2 xl
    xltab = nc.dram_tensor("xltab", [NC * SLH, P], BF, addr_space="Shared")
    fpk_d = nc.dram_tensor("fpk_d", [D, FC], F32)              # local partials
    fpkg = nc.dram_tensor("fpkg", [NC * D, FC], F32, addr_space="Shared")

    rg = [list(range(NC))]

    with tile.TileContext(nc) as tc:
        with tc.tile_pool(name="cst", bufs=1) as cst, \
             tc.tile_pool(name="pb", bufs=2, space="PSUM") as ps_b_pool, \
             tc.tile_pool(name="win", bufs=2, space="PSUM") as ps_win_pool, \
             tc.tile_pool(name="acc", bufs=1, space="PSUM") as ps_acc_pool, \
             tc.tile_pool(name="xsm", bufs=1, space="PSUM") as ps_xsm_pool, \
             tc.tile_pool(name="gat", bufs=3) as gatp, \
             tc.tile_pool(name="wrk", bufs=3) as wrk:

            def load_const(t, shape, dtype):
                s = cst.tile(shape, dtype, tag=t.name)
                nc.sync.dma_start(s[:], t[:])
                return s

            # phase-B-critical consts first (HWDGE drains in FIFO order)
            xnm_t = load_const(t_xnm, [P, W * CHX], BF)
            wlr_t = load_const(t_wlr, [CHX, 2 * D], BF)
            id_t = load_const(t_id, [P, P], BF)
            gmat_t = load_const(t_gmat, [P, W * GS2], BF)
            dstrel_t = load_const(t_dstrel, [P, T8], BF)
            par_t = load_const(t_par, [P, T8], BF)
            eac_t = load_const(t_eac, [P, T8], BF)
            attc_t = load_const(t_attc, [P, 8 * D], BF)
            webc_t = load_const(t_webc, [P, D], BF)
            iotac_t = load_const(t_iotac, [P, P], BF)
            wres_t = load_const(t_wres, [CHX, D], BF)
            misc_t = load_const(t_misc, [D, 8], F32)
            w1_t = load_const(t_w1, [D + 1, D], BF)
            w2_t = load_const(t_w2, [D + 1, 2], BF)
            # gather idx streams: ship [16, L16], replicate to the
            # [128, L16] wrapped format dma_gather expects
            dsti_t = cst.tile([P, L16], I16, tag="dsti")
            srci_t = cst.tile([P, L16], I16, tag="srci")
            for r in range(8):
                nc.sync.dma_start(dsti_t[16 * r:16 * (r + 1), :], t_dsti[:])
                nc.sync.dma_start(srci_t[16 * r:16 * (r + 1), :], t_srci[:])

            xnm_v = xnm_t[:].rearrange("p (w c) -> p w c", w=W)
            gmat_v = gmat_t[:].rearrange("p (w g) -> p w g", w=W)
            xlslab_v = xlslab[:].rearrange("r (h f) -> r h f", h=2)

            # f32 copies of streams used as tensor_scalar scalar operands
            dstrel_f = cst.tile([P, T8], F32, tag="dstrel_f")
            nc.scalar.activation(dstrel_f[:], dstrel_t[:], AF.Copy)
            eac_f = cst.tile([P, T8], F32, tag="eac_f")
            nc.scalar.activation(eac_f[:], eac_t[:], AF.Copy)

            ps_stats = ps_acc_pool.tile([P, GS2], F32, tag="stats")
            ps_xsum = ps_xsm_pool.tile([CHX, GS2], F32, tag="xsum")

            # phase B: per-window transpose + [xl|xr] projection + tables
            for w in range(W):
                ps_tr = ps_b_pool.tile([CHX, P], F32, tag="pb", name=f"tr{w}")
                nc.tensor.matmul(ps_tr[:], xnm_v[:, w, :], id_t[:],
                                 start=True, stop=True)
                sb_tr = wrk.tile([CHX, P], BF, tag="tr", name=f"trs{w}")
                nc.scalar.activation(sb_tr[:], ps_tr[:], AF.Copy)
                ps_lr = ps_b_pool.tile([P, 2 * D], F32, tag="pb", name=f"lr{w}")
                nc.tensor.matmul(ps_lr[:], sb_tr[:], wlr_t[:],
                                 start=True, stop=True)
                sb_lr = wrk.tile([P, 2 * D], BF, tag="lr", name=f"lrs{w}")
                nc.scalar.activation(sb_lr[:], ps_lr[:], AF.Copy)
                nc.sync.dma_start(xrtab[w * P:(w + 1) * P, :], sb_lr[:])
                # pack-2 xl half-slab write (nodes n, n+SLH share a 256B row)
                r0 = (w * P) % SLH
                h0 = (w * P) // SLH
                if r0 + P <= SLH:
                    nc.sync.dma_start(xlslab_v[r0:r0 + P, h0, :],
                                      sb_lr[:, 0:D])
                else:
                    mcut = SLH - r0
                    nc.sync.dma_start(xlslab_v[r0:SLH, h0, :],
                                      sb_lr[0:mcut, 0:D])
                    nc.sync.dma_start(xlslab_v[0:P - mcut, h0 + 1, :],
                                      sb_lr[mcut:P, 0:D])
                nc.tensor.matmul(ps_xsum[:], xnm_v[:, w, :], gmat_v[:, w, :],
                                 start=(w == 0), stop=(w == W - 1),
                                 skip_group_check=True)

            nc.gpsimd.collective_compute(
                "AllGather", AL.bypass, replica_groups=rg,
                ins=[xlslab[:].opt()], outs=[xltab[:].opt()])

            # phase C: edge loop (scatter matmuls skewed one group behind)
            win_tiles = {}
            pend = []

            def emit_scatter(gq, oh_q, msg_q, gl_q, lg_q):
                sb_exq = wrk.tile([P, 8, D], BF, tag="exq", name=f"exq{gq}")
                nc.scalar.activation(
                    sb_exq[:].rearrange("p c (h k) -> p c h k", k=OUT_C),
                    msg_q[:, :, D:D + HEADS].unsqueeze(3).to_broadcast(
                        [P, 8, HEADS, OUT_C]),
                    AF.Copy)
                nc.vector.tensor_tensor(
                    out=msg_q[:, :, 0:D], in0=gl_q[:], in1=sb_exq[:],
                    op=AL.mult)
                flush = []
                for c8 in range(8):
                    c = gq * 8 + c8
                    w = min(c // CPW, W - 1)
                    if w not in win_tiles:
                        win_tiles[w] = ps_win_pool.tile([P, D + HEADS], F32,
                                                        tag="win", name=f"win{gq}_{w}")
                    first = (c % CPW == 0) and c < W * CPW
                    last = (c == (w + 1) * CPW - 1) if w < W - 1 else (c == T8 - 1)
                    nc.tensor.matmul(win_tiles[w][:], oh_q[:, c8, :],
                                     msg_q[:, c8, :], start=first, stop=last,
                                     skip_group_check=True)
                    if last:
                        flush.append(w)
                return flush

            def do_flush(flush):
                for w in flush:
                    ps_w = win_tiles.pop(w)
                    sb_den = wrk.tile([P, HEADS], F32, tag="den", name=f"den{w}")
                    nc.vector.tensor_scalar(sb_den[:], ps_w[:, D:D + HEADS],
                                            1e-20, None, AL.add)
                    sb_rd = wrk.tile([P, HEADS], F32, tag="rd", name=f"rd{w}")
                    nc.vector.reciprocal(sb_rd[:], sb_den[:])
                    sb_hh2 = wrk.tile([P, 2 * D], BF, tag="hh2", name=f"hh2{w}")
                    nc.vector.tensor_tensor(
                        out=sb_hh2[:, 0:D].rearrange("p (h k) -> p h k", k=OUT_C),
                        in0=ps_w[:, 0:D].rearrange("p (h k) -> p h k", k=OUT_C),
                        in1=sb_rd[:].unsqueeze(2).to_broadcast([P, HEADS, OUT_C]),
                        op=AL.mult)
                    nc.scalar.activation(sb_hh2[:, D:2 * D], sb_hh2[:, 0:D],
                                         AF.Square)
                    nc.tensor.matmul(ps_stats[:], sb_hh2[:], gmat_v[:, w, :],
                                     start=(w == 0), stop=(w == W - 1),
                                     skip_group_check=True)

            for g in range(NG):
                grx = gatp.tile([P, 8, P], BF, tag="grx")
                nc.gpsimd.dma_gather(
                    out_ap=grx[:], in_ap=xrtab[:],
                    idxs_ap=dsti_t[:, g * 64:(g + 1) * 64],
                    num_idxs=1024, num_idxs_reg=1024, elem_size=P,
                    queue_num=0)
                grl = gatp.tile([P, 8, P], BF, tag="grl")
                nc.gpsimd.dma_gather(
                    out_ap=grl[:], in_ap=xltab[:],
                    idxs_ap=srci_t[:, g * 64:(g + 1) * 64],
                    num_idxs=1024, num_idxs_reg=1024, elem_size=P,
                    queue_num=1)

                # xl_sel = lo + par*(hi - lo)   (pack-2 column-half select)
                sb_diff = wrk.tile([P, 8, D], BF, tag="diff")
                nc.vector.tensor_tensor(out=sb_diff[:], in0=grl[:, :, D:2 * D],
                                        in1=grl[:, :, 0:D], op=AL.subtract)
                sb_pd = wrk.tile([P, 8, D], BF, tag="pd")
                nc.vector.tensor_tensor(
                    out=sb_pd[:], in0=sb_diff[:],
                    in1=par_t[:, g * 8:(g + 1) * 8].unsqueeze(2).to_broadcast(
                        [P, 8, D]),
                    op=AL.mult)
                sb_gl = wrk.tile([P, 8, D], BF, tag="gl")
                nc.vector.tensor_tensor(out=sb_gl[:], in0=sb_pd[:],
                                        in1=grl[:, :, 0:D], op=AL.add)

                # s = xl_sel + xr[dst] + ea*We
                sb_s1 = wrk.tile([P, 8, D], BF, tag="s1")
                nc.vector.tensor_tensor(out=sb_s1[:], in0=sb_gl[:],
                                        in1=grx[:, :, D:2 * D], op=AL.add)
                sb_eaw = wrk.tile([P, 8, D], BF, tag="eaw")
                for c8 in range(8):
                    nc.vector.tensor_scalar(
                        sb_eaw[:, c8, :], webc_t[:],
                        eac_f[:, g * 8 + c8:g * 8 + c8 + 1], None, AL.mult)
                sb_sp = wrk.tile([P, 8, D], BF, tag="sp")
                nc.vector.tensor_tensor(out=sb_sp[:], in0=sb_s1[:],
                                        in1=sb_eaw[:], op=AL.add)

                sb_t = wrk.tile([P, 8, D], BF, tag="t")
                nc.scalar.activation(sb_t[:], sb_sp[:], AF.Prelu,
                                     alpha=NEG_SLOPE)
                if pend:
                    _, _, pmsg, _, plg = pend[-1]
                    nc.scalar.activation(pmsg[:, :, D:D + HEADS], plg[:], AF.Exp)

                sb_u = wrk.tile([P, 8, D], BF, tag="u")
                nc.vector.tensor_tensor(
                    out=sb_u[:], in0=sb_t[:],
                    in1=attc_t[:].rearrange("p (c f) -> p c f", c=8),
                    op=AL.mult)
                sb_lg = wrk.tile([P, 8, HEADS], F32, tag="lg")
                nc.vector.tensor_reduce(
                    out=sb_lg[:],
                    in_=sb_u[:].rearrange("p c (h k) -> p c h k", k=OUT_C),
                    axis=AX.X, op=AL.add)
                sb_msg = wrk.tile([P, 8, D + HEADS], BF, tag="msg")

                oh_t = wrk.tile([P, 8, P], BF, tag="oh")
                for c8 in range(8):
                    nc.vector.tensor_scalar(
                        oh_t[:, c8, :], iotac_t[:],
                        dstrel_f[:, g * 8 + c8:g * 8 + c8 + 1], None,
                        AL.is_equal)

                pend.append((g, oh_t, sb_msg, sb_gl, sb_lg))
                if len(pend) > 1:
                    do_flush(emit_scatter(*pend.pop(0)))

            while pend:
                _, _, pmsg, _, plg = pend[0]
                nc.scalar.activation(pmsg[:, :, D:D + HEADS], plg[:], AF.Exp)
                do_flush(emit_scatter(*pend.pop(0)))

            # phase D: pack local partials, AllGather, fused BN + MLP tail
            sb_sloc = wrk.tile([P, 1], F32, tag="sloc")
            nc.vector.tensor_reduce(out=sb_sloc[:], in_=ps_stats[:, 0:GSLOT],
                                    axis=AX.X, op=AL.add)
            sb_hdiv = wrk.tile([D, GSLOT], F32, tag="hdiv")
            nc.scalar.activation(sb_hdiv[:], ps_stats[0:D, GSLOT:GS2], AF.Copy)
            sb_xdiv = wrk.tile([CHX, GSLOT], BF, tag="xdiv")
            nc.scalar.activation(sb_xdiv[:], ps_xsum[:, GSLOT:GS2], AF.Copy)
            ps_res = ps_b_pool.tile([D, GSLOT], F32, tag="pb", name="res")
            nc.tensor.matmul(ps_res[:], wres_t[:], sb_xdiv[:], start=True,
                             stop=True)
            sb_res = wrk.tile([D, GSLOT], F32, tag="res")
            nc.scalar.activation(sb_res[:], ps_res[:], AF.Copy)

            nc.sync.dma_start(fpk_d[:, 0:1], sb_sloc[0:D, :])
            nc.sync.dma_start(fpk_d[:, 1:2], sb_sloc[D:2 * D, :])
            nc.sync.dma_start(fpk_d[:, 2:2 + GSLOT], sb_hdiv[:])
            nc.sync.dma_start(fpk_d[:, 2 + GSLOT:FC], sb_res[:])

            nc.gpsimd.collective_compute(
                "AllGather", AL.bypass, replica_groups=rg,
                ins=[fpk_d[:].opt()], outs=[fpkg[:].opt()])

            fv = fpkg[:].rearrange("(k p) c -> p k c", k=NC)   # [64, 8, FC]
            sb_s8 = wrk.tile([D, 2, NC], F32, tag="s8")
            nc.sync.dma_start(sb_s8[:, 0, :], fv[:, :, 0])
            nc.sync.dma_start(sb_s8[:, 1, :], fv[:, :, 1])
            sb_pool = wrk.tile([D, GALL], F32, tag="pool")
            nc.sync.dma_start(
                sb_pool[:].rearrange("p (k j) -> p k j", k=NC),
                fv[:, :, 2:2 + GSLOT])
            sb_resg = wrk.tile([D, GALL], F32, tag="resg")
            nc.sync.dma_start(
                sb_resg[:].rearrange("p (k j) -> p k j", k=NC),
                fv[:, :, 2 + GSLOT:FC])

            sh2 = wrk.tile([D, 2], F32, tag="sh2")
            nc.vector.tensor_reduce(out=sh2[:], in_=sb_s8[:], axis=AX.X,
                                    op=AL.add)
            mu = wrk.tile([D, 1], F32, tag="mu")
            nc.scalar.activation(mu[:], sh2[:, 0:1], AF.Copy, scale=1.0 / N)
            e2 = wrk.tile([D, 1], F32, tag="e2")
            nc.scalar.activation(e2[:], sh2[:, 1:2], AF.Copy, scale=1.0 / N)
            mu2 = wrk.tile([D, 1], F32, tag="mu2")
            nc.scalar.activation(mu2[:], mu[:], AF.Square)
            var = wrk.tile([D, 1], F32, tag="var")
            nc.vector.tensor_tensor(out=var[:], in0=e2[:], in1=mu2[:],
                                    op=AL.subtract)
            sd = wrk.tile([D, 1], F32, tag="sd")
            nc.scalar.activation(sd[:], var[:], AF.Sqrt, bias=misc_t[:, 3:4])
            rsd = wrk.tile([D, 1], F32, tag="rsd")
            nc.vector.reciprocal(rsd[:], sd[:])
            A = wrk.tile([D, 1], F32, tag="A")
            nc.vector.tensor_tensor(out=A[:], in0=misc_t[:, 0:1], in1=rsd[:],
                                    op=AL.mult)
            tmp2 = wrk.tile([D, 1], F32, tag="tmp2")
            nc.vector.tensor_tensor(out=tmp2[:], in0=A[:], in1=mu[:], op=AL.mult)
            B = wrk.tile([D, 1], F32, tag="B")
            nc.vector.tensor_tensor(out=B[:], in0=misc_t[:, 1:2], in1=tmp2[:],
                                    op=AL.subtract)

            pooled = wrk.tile([D, GALL], F32, tag="pooled")
            nc.vector.tensor_scalar(pooled[:], sb_pool[:], A[:], B[:],
                                    AL.mult, AL.add)
            zr = wrk.tile([D + 1, GALL], BF, tag="zr")
            nc.vector.memset(zr[D:D + 1, :], 1.0)
            nc.vector.tensor_tensor(out=zr[0:D, :], in0=pooled[:],
                                    in1=sb_resg[:], op=AL.add)
            ps_z = ps_b_pool.tile([D, GALL], F32, tag="pb", name="z")
            nc.tensor.matmul(ps_z[:], w1_t[:], zr[:], start=True, stop=True)
            z2 = wrk.tile([D + 1, GALL], BF, tag="z2")
            nc.vector.memset(z2[D:D + 1, :], 1.0)
            nc.scalar.activation(z2[0:D, :], ps_z[:], AF.Relu)
            ps_o = ps_b_pool.tile([2, GALL], F32, tag="pb", name="o")
            nc.tensor.matmul(ps_o[:], w2_t[:], z2[:], start=True, stop=True)
            sb_o = wrk.tile([2, GALL], F32, tag="out")
            nc.scalar.activation(sb_o[:], ps_o[:], AF.Copy)
            nc.sync.dma_start(t_out[:], sb_o[:])

    nc.compile()
    return nc


# --------------------------------------------------------------------------
# entry point
# --------------------------------------------------------------------------

def kernel(**inputs):
    meta, in_maps = _prep(inputs)
    key = ("main", meta["CHX"], meta["W"], meta["CPW"], meta["T8"])
    if key not in _prog_cache:
        _prog_cache[key] = _build(meta)
    nc_main = _prog_cache[key]

    NC = meta["NC"]
    global LAST_EXEC_NS
    from concourse.bass_utils import run_bass_kernel_spmd
    import time as _time
    _t0 = _time.time()
    r1 = run_bass_kernel_spmd(nc_main, in_maps, list(range(NC)), **_RUN_KW)
    _t1 = _time.time()
    LAST_EXEC_NS = [getattr(r1, "exec_time_ns", None) or int((_t1 - _t0) * 1e9)]
    t_out = r1.results[0]["t_out"]

    G = meta["G"]
    gb = meta["gb"]
    out = np.zeros((G, 2), np.float32)
    for g in range(G):
        k = int(np.searchsorted(gb, g, side="right")) - 1
        slot = g - int(gb[k])
        out[g] = t_out[:, k * GSLOT + slot]
    return out


_RUN_KW = {}
LAST_EXEC_NS = None
